# revision 21
# baseline (speedup 1.0000x reference)
"""Trainium2 Bass kernel for the capsule-routing layer (nn_Caps_Layer).

Full inputs: x [32, 512, 768] f32, W [1, 768, 512] f32.
Output: [32, 16, 32] f32.

Strategy: data-parallel over batch across 8 NeuronCores (4 batches/core),
inputs converted to bf16 on the host (halves the HBM traffic; rel-err
budget 2e-2 >> bf16's ~5e-3).

Per core the routing loop is algebraically factored so u_hat [S, N*C]
is never materialized:
    iter0:   m0[(nc)]   = xsum @ W             (xsum = col-sum of x)
    V[d,n]   = sum_c W[d,(n c)] * mnorm[n,c]   (Wt-chunk @ Mblk, ap=16)
    b[s,n]   = x @ V                           (xT-chunk @ V,     ap=16)
    c        = softmax_n(b)
    G[n,d]   = c^T @ x                         (x-chunk @ c,      ap=16)
    m[(nc)]  = diag_n(W^T G)                   (W-chunk @ G^T,    ap=16)
    squash: inv = exp(-0.5 ln(|m|^2 + eps))    (one ACT table: exp+ln)
All routing matmuls keep the tiny capsule dim (16) as the moving side, so
PE streaming cost is ~16 cycles/matmul; the only large PE work is the
x-transposes (needed for the d-major contraction in b = x @ V).
"""
import numpy as np
import concourse.bass as bass
import concourse.mybir as mybir
import concourse.tile as tile
from concourse import bacc
from concourse.bass import ts, ds
from concourse.bass_utils import run_bass_kernel_spmd

F32 = mybir.dt.float32
U32 = mybir.dt.uint32
BF16 = mybir.dt.bfloat16
AF = mybir.ActivationFunctionType
AX = mybir.AxisListType
OP = mybir.AluOpType

NCORES = 8
B, S, D = 32, 512, 768
N, C = 16, 32
NC = N * C            # 512
BL = B // NCORES      # 4 batches per core
EPS = 1e-7
SCN = S // 128        # 4 s-chunks
DCN = D // 128        # 6 d-chunks
KCN = NC // 128       # 4 nc-chunks
ROUTINGS = 3
PE_WARM = 0   # dummy PE transposes chained to hold the p-state at 2.4 GHz

# const tile column layout (all bf16)
CID = 0               # [128, 128] identity (PE transposes)
CMASK = 128           # [128, 256] diag mask[(nl,c), (b,kc,n)] = (n == 4*kc+nl)
CSEL = 384            # [128, 4]   sel[p, j] = (p//32 == j)
CONE = 388            # [128, 1]   ones
CBS = 392             # [128, 128] bsel[k, m] = (k//32 == m//32)
CONW = 520


def _build_module():
    nc = bacc.Bacc("TRN2", target_bir_lowering=False, num_devices=NCORES)
    X = nc.dram_tensor("x", [BL, S, D], BF16, kind="ExternalInput")
    W = nc.dram_tensor("w", [D, NC], BF16, kind="ExternalInput")
    CON = nc.dram_tensor("consts", [128, CONW], BF16, kind="ExternalInput")
    OUT = nc.dram_tensor("out", [BL, N, C], F32, kind="ExternalOutput")

    cp_flip = [0]

    with tile.TileContext(nc) as tc:
        with (
            tc.tile_pool(name="const", bufs=1) as pc,
            tc.tile_pool(name="rt", bufs=2) as prt,
            tc.tile_pool(name="pmm", bufs=1, space="PSUM") as pmm,
            tc.tile_pool(name="ptr", bufs=2, space="PSUM") as ptr,
        ):
            def cpd(dst, src):
                nc.vector.tensor_copy(dst, src)

            def cpa(dst, src):
                nc.scalar.copy(dst, src)

            # ---- persistent tiles ----
            con = pc.tile([128, CONW], BF16, tag="con")
            wsb = pc.tile([128, DCN, NC], BF16, tag="w")
            wtsb = pc.tile([128, KCN, D], BF16, tag="wt")
            xsumb = pc.tile([128, BL * DCN], BF16, tag="xsum")
            epst = pc.tile([128, 1], F32, tag="eps")
            nc.vector.memset(epst[:], EPS)
            magict = pc.tile([128, 16], U32, tag="magic")
            nc.vector.memset(magict[:], 0x5F3759DF)

            def prefetch_act(func):
                # dummy [1,1] activation hoists the ACT table load early
                dum = prt.tile([1, 1], F32, tag="dum")
                nc.scalar.activation(dum[:], epst[0:1, :], func)

            # consts ride the ACT queue; x batches + W ride the SP queue.
            # W sits between x1 and x2 so WT transposes clear the PE early;
            # the last batch arrives in s-chunks so stage A can track it.
            nc.scalar.dma_start(con[:], CON[:, :])
            xbs = [pc.tile([128, SCN, D], BF16, tag=f"xb{b}", name=f"xb_{b}")
                   for b in range(BL)]
            nc.sync.dma_start(
                xbs[0][:], X[0, :, :].rearrange("(sc p) d -> p sc d", p=128)
            )
            nc.sync.dma_start(
                wsb[:], W[:, :].rearrange("(dc p) n -> p dc n", p=128)
            )
            for b in (1, 2):
                nc.sync.dma_start(
                    xbs[b][:],
                    X[b, :, :].rearrange("(sc p) d -> p sc d", p=128),
                )
            for sc in range(SCN):
                nc.sync.dma_start(
                    xbs[3][:, sc, :], X[3, ds(sc * 128, 128), :]
                )
            prefetch_act(AF.Exp)

            ident = con[:, CID:CID + 128]

            # ---- stage A: xT + xsum per batch; WT between b2 and b3 so
            # the W transposes fill the DMA wait for the last batch ----
            pxs = pmm.tile([128, BL * DCN], F32, tag="seq")
            xts = [pc.tile([128, DCN, S], BF16, tag=f"xt{b}", name=f"xt_{b}")
                   for b in range(BL)]

            def stage_a(b):
                xb = xbs[b]
                xt = xts[b]
                for j in range(DCN // 2):
                    pxt = ptr.tile([128, 2 * S], BF16, tag="tr")
                    for h in range(2):
                        dc = 2 * j + h
                        for sc in range(SCN):
                            nc.tensor.transpose(
                                pxt[:, ds(h * S + sc * 128, 128)],
                                xb[:, sc, ds(dc * 128, 128)],
                                ident,
                            )
                        for sc in range(SCN):
                            nc.tensor.matmul(
                                pxs[:, ds(b * DCN + dc, 1)],
                                xb[:, sc, ds(dc * 128, 128)],
                                con[:, CONE:CONE + 1],
                                start=(sc == 0),
                                stop=(sc == SCN - 1),
                            )
                    eng = cpa if j == 1 else cpd
                    eng(xt[:, 2 * j:2 * j + 2, :].rearrange(
                        "p a b -> p (a b)"), pxt[:])
                cpd(xsumb[:, ds(b * DCN, DCN)], pxs[:, ds(b * DCN, DCN)])

            def stage_a3():
                # last batch: transposes emitted s-chunk-major (chunks arrive
                # via four DMAs), xsum evacuated before the pair tiles so
                # iter0 can start the moment the last chunk lands
                xb, xt = xbs[3], xts[3]
                p3 = [ptr.tile([128, 2 * S], BF16, tag="tr3", bufs=3,
                               name=f"tr3_{j}") for j in range(3)]
                for sc in range(SCN):
                    for dc in range(DCN):
                        nc.tensor.transpose(
                            p3[dc // 2][:, ds((dc % 2) * S + sc * 128, 128)],
                            xb[:, sc, ds(dc * 128, 128)],
                            ident,
                        )
                for dc in range(DCN):
                    for sc in range(SCN):
                        nc.tensor.matmul(
                            pxs[:, ds(3 * DCN + dc, 1)],
                            xb[:, sc, ds(dc * 128, 128)],
                            con[:, CONE:CONE + 1],
                            start=(sc == 0),
                            stop=(sc == SCN - 1),
                        )
                cpd(xsumb[:, ds(3 * DCN, DCN)], pxs[:, ds(3 * DCN, DCN)])
                for j in range(3):
                    eng = cpa if j == 1 else cpd
                    eng(xt[:, 2 * j:2 * j + 2, :].rearrange(
                        "p a b -> p (a b)"), p3[j][:])

            for b in range(2):
                stage_a(b)
            for kc in range(KCN):
                ptw = ptr.tile([128, 2 * S], BF16, tag="tr")
                for dc in range(DCN):
                    nc.tensor.transpose(
                        ptw[:, ts(dc, 128)],
                        wsb[:, dc, ds(kc * 128, 128)],
                        ident,
                    )
                (cpd if kc % 2 == 0 else cpa)(wtsb[:, kc, :], ptw[:, 0:768])
            stage_a(2)
            stage_a3()

            # PE warmer: a serial chain of dummy transposes keeps the PE
            # p-state ramped through the routing phase so the small matmul
            # groups run at full clock.
            if PE_WARM:
                pwm = pmm.tile([128, 128], BF16, tag="warm")
                for _ in range(PE_WARM):
                    nc.tensor.transpose(pwm[:], con[:, 0:128], ident)

            # ---- routing ----
            maskr = con[:, CMASK:CMASK + BL * KCN * N]

            def squash(pot, src_cols, it):
                """pot: psum [128, (b kc[ n])] -> returns mnorm tile.
                src_cols=1 for iter0 (pot is [128, (b kc)] = m directly)."""
                small = prt.tile([128, 16], F32, tag="m", name=f"m{it}")
                if src_cols == 1:
                    nc.vector.tensor_copy(small[:], pot[:])
                    m = small
                else:
                    pm = prt.tile([128, BL * KCN * N], F32, tag="pm")
                    nc.vector.tensor_mul(pm[:], pot[:], maskr)
                    nc.vector.tensor_reduce(
                        small[:],
                        pm[:].rearrange("p (g n) -> p g n", g=BL * KCN),
                        axis=AX.X,
                        op=OP.add,
                    )
                    m = small
                sq = prt.tile([128, 16], BF16, tag="sq", name=f"sq{it}")
                nc.vector.tensor_mul(sq[:], m[:], m[:])
                # fused capsule-group sum + broadcast: bsel[k,m]=(k//32==m//32)
                pnsq = pmm.tile([128, 16], F32, tag="seq", name=f"nsq{it}")
                nc.tensor.matmul(
                    pnsq[:],
                    con[:, CBS:CBS + 128],
                    sq[:],
                    start=True,
                    stop=True,
                )
                # rsqrt on DVE only (bit trick + 1 Newton step); keeps the
                # ACT table pinned to Exp for the whole kernel. nsq is
                # O(10..100) here so the reference's +eps is a no-op in bf16.
                y0u = prt.tile([128, 16], U32, tag="y0u", name=f"y0u{it}")
                nc.vector.tensor_scalar(
                    y0u[:], pnsq[:].bitcast(U32), 1, None,
                    OP.logical_shift_right,
                )
                nc.vector.tensor_sub(y0u[:], magict[:], y0u[:])
                y0f = y0u[:].bitcast(F32)
                t1 = prt.tile([128, 16], F32, tag="nt1", name=f"nt1{it}")
                nc.vector.tensor_mul(t1[:], pnsq[:], y0f)
                nc.vector.tensor_mul(t1[:], t1[:], y0f)
                nc.vector.tensor_scalar(t1[:], t1[:], -0.5, 1.5, OP.mult,
                                        OP.add)
                pinv = prt.tile([128, 16], F32, tag="rsq", name=f"rsq{it}")
                nc.vector.tensor_mul(pinv[:], y0f, t1[:])
                if it == ROUTINGS - 1:
                    mnorm = prt.tile([128, 16], BF16, tag="mnf", name="mnf")
                    nc.vector.tensor_mul(mnorm[:], m[:], pinv[:])
                    return mnorm
                # pre-scatter m over the diag mask on the idle GPSIMD engine
                # while the DVE rsqrt chain runs; fold pinv in afterwards
                mm1 = prt.tile([128, BL * KCN * N], BF16, tag="mm1",
                               name=f"mm1_{it}")
                m_bc = bass.AP(
                    tensor=m.tensor,
                    offset=m.offset,
                    ap=[m.ap[0], [1, BL * KCN], [0, N]],
                )
                nc.gpsimd.tensor_mul(
                    mm1[:].rearrange("p (g n) -> p g n", g=BL * KCN),
                    m_bc,
                    maskr.rearrange("p (g n) -> p g n", g=BL * KCN),
                )
                mblk = prt.tile([128, BL * KCN * N], BF16, tag="mblk",
                                name=f"mblk{it}")
                pi_bc = bass.AP(
                    tensor=pinv.tensor,
                    offset=pinv.offset,
                    ap=[pinv.ap[0], [1, BL * KCN], [0, N]],
                )
                nc.vector.tensor_mul(
                    mblk[:].rearrange("p (g n) -> p g n", g=BL * KCN),
                    mm1[:].rearrange("p (g n) -> p g n", g=BL * KCN),
                    pi_bc,
                )
                return mblk

            def v_and_b(mblk, it, mid=None):
                """V -> b (psum), half-batch interleaved with fully
                independent tiles per half so the evacuations overlap the
                other half's matmuls."""
                HB = BL * DCN * N // 2
                vsbs = []
                for half in range(2):
                    pv = pmm.tile([128, HB], F32, tag=f"big{half}",
                                  name=f"pv{it}_{half}")
                    for b in (0, 1) if half == 0 else (2, 3):
                        for dc in range(DCN):
                            col = ((b % 2) * DCN + dc) * N
                            for kc in range(KCN):
                                nc.tensor.matmul(
                                    pv[:, ds(col, N)],
                                    wtsb[:, kc, ds(dc * 128, 128)],
                                    mblk[:, ds((b * KCN + kc) * N, N)],
                                    start=(kc == 0),
                                    stop=(kc == KCN - 1),
                                )
                    vsb = prt.tile([128, HB], BF16, tag=f"vsb{half}",
                                   name=f"vsb{it}_{half}")
                    (nc.scalar.copy if half == 0
                     else nc.vector.tensor_copy)(vsb[:], pv[:])
                    vsbs.append(vsb)
                if mid is not None:
                    mid()
                pb = pmm.tile([128, BL * SCN * N], F32, tag="seq")
                for b in range(BL):
                    for sc in range(SCN):
                        for dc in range(DCN):
                            nc.tensor.matmul(
                                pb[:, ds((b * SCN + sc) * N, N)],
                                xts[b][:, dc, ds(sc * 128, 128)],
                                vsbs[b // 2][:, ds(((b % 2) * DCN + dc) * N, N)],
                                start=(dc == 0),
                                stop=(dc == DCN - 1),
                            )
                return pb

            # iter 0: uniform routing weights -> m0 = xsum @ W (diag blocks)
            pot0 = pmm.tile([128, BL * KCN], F32, tag="seq")
            for b in range(BL):
                for kc in range(KCN):
                    for dc in range(DCN):
                        nc.tensor.matmul(
                            pot0[:, ds(b * KCN + kc, 1)],
                            wsb[:, dc, ds(kc * 128, 128)],
                            xsumb[:, ds(b * DCN + dc, 1)],
                            start=(dc == 0),
                            stop=(dc == DCN - 1),
                        )
            mblk0 = squash(pot0, 1, 0)
            pb = v_and_b(mblk0, 0)

            for it in range(1, ROUTINGS):
                # softmax over n
                expb = prt.tile([128, BL * SCN * N], F32, tag="expb",
                                name=f"expb{it}")
                nc.scalar.activation(expb[:], pb[:], AF.Exp)
                zsum = prt.tile([128, BL * SCN], F32, tag="zsum",
                                name=f"zsum{it}")
                nc.vector.tensor_reduce(
                    zsum[:],
                    expb[:].rearrange("p (g n) -> p g n", g=BL * SCN),
                    axis=AX.X,
                    op=OP.add,
                )
                zrec = prt.tile([128, BL * SCN], F32, tag="zrec",
                                name=f"zrec{it}")
                nc.vector.reciprocal(zrec[:], zsum[:])
                cw = prt.tile([128, BL * SCN * N], BF16, tag="cw",
                              name=f"cw{it}")
                zr_bc = bass.AP(
                    tensor=zrec.tensor,
                    offset=zrec.offset,
                    ap=[zrec.ap[0], [1, BL * SCN], [0, N]],
                )
                nc.vector.tensor_mul(
                    cw[:].rearrange("p (g n) -> p g n", g=BL * SCN),
                    expb[:].rearrange("p (g n) -> p g n", g=BL * SCN),
                    zr_bc,
                )
                # G^T[d, n] per (b, dc), then outT[(nc), n] per (b, kc);
                # half-batch interleaved with independent tiles per half
                HB = BL * DCN * N // 2
                gsbs = []
                for half in range(2):
                    pg = pmm.tile([128, HB], F32, tag=f"big{half}",
                                  name=f"gp{it}_{half}")
                    for b in (0, 1) if half == 0 else (2, 3):
                        for dc in range(DCN):
                            col = ((b % 2) * DCN + dc) * N
                            for sc in range(SCN):
                                nc.tensor.matmul(
                                    pg[:, ds(col, N)],
                                    xbs[b][:, sc, ds(dc * 128, 128)],
                                    cw[:, ds((b * SCN + sc) * N, N)],
                                    start=(sc == 0),
                                    stop=(sc == SCN - 1),
                                )
                    gsb = prt.tile([128, HB], BF16, tag=f"gsb{half}",
                                   name=f"gsb{it}_{half}")
                    (nc.scalar.copy if half == 0
                     else nc.vector.tensor_copy)(gsb[:], pg[:])
                    gsbs.append(gsb)
                pot = pmm.tile([128, BL * KCN * N], F32, tag="seq",
                               name=f"potp{it}")
                for b in range(BL):
                    for kc in range(KCN):
                        for dc in range(DCN):
                            nc.tensor.matmul(
                                pot[:, ds((b * KCN + kc) * N, N)],
                                wsb[:, dc, ds(kc * 128, 128)],
                                gsbs[b // 2][:, ds(((b % 2) * DCN + dc) * N, N)],
                                start=(dc == 0),
                                stop=(dc == DCN - 1),
                            )
                mnorm = squash(pot, N, it)
                if it < ROUTINGS - 1:
                    pb = v_and_b(mnorm, it)

            # final output: transpose to [(b kc), (nl c)] so each DMA
            # descriptor is a 512-byte contiguous DRAM run
            pfin = pmm.tile([16, 128], BF16, tag="seq")
            nc.tensor.transpose(pfin[:], mnorm[:], ident)
            fsb = prt.tile([16, 128], F32, tag="fsb")
            nc.scalar.copy(fsb[:], pfin[:])
            nc.sync.dma_start(
                OUT.rearrange("b (kc nl) c -> (b kc) (nl c)", kc=KCN, nl=4),
                fsb[:],
            )

    nc.compile()
    return nc


def _make_consts():
    import ml_dtypes
    con = np.zeros((128, CONW), dtype=np.float32)
    con[:, CID:CID + 128] = np.eye(128, dtype=np.float32)
    p = np.arange(128)
    for b in range(BL):
        for kc in range(KCN):
            for n in range(N):
                con[:, CMASK + (b * KCN + kc) * N + n] = (n == 4 * kc + p // 32)
    for j in range(4):
        con[:, CSEL + j] = (p // 32 == j)
    con[:, CONE] = 1.0
    con[:, CBS:CBS + 128] = (p[:, None] // 32 == p[None, :] // 32)
    return con.astype(ml_dtypes.bfloat16)


_NC_CACHE = []


def kernel(x: np.ndarray, W: np.ndarray) -> np.ndarray:
    import ml_dtypes
    assert x.shape == (B, S, D) and W.shape == (1, D, NC)
    if not _NC_CACHE:
        _NC_CACHE.append(_build_module())
    nc = _NC_CACHE[0]
    con = _make_consts()
    w2 = np.ascontiguousarray(W[0]).astype(ml_dtypes.bfloat16)
    xb = x.astype(ml_dtypes.bfloat16)
    in_maps = []
    for i in range(NCORES):
        m = {
            "x": np.ascontiguousarray(xb[i * BL:(i + 1) * BL]),
            "w": w2,
            "consts": con,
        }
        in_maps.append(m)
    res = run_bass_kernel_spmd(nc, in_maps, list(range(NCORES)))
    out = np.concatenate([res.results[i]["out"] for i in range(NCORES)], axis=0)
    return out.astype(np.float32)


# revision 22
# speedup vs baseline: 1.0019x; 1.0019x over previous
"""Trainium2 Bass kernel for the capsule-routing layer (nn_Caps_Layer).

Full inputs: x [32, 512, 768] f32, W [1, 768, 512] f32.
Output: [32, 16, 32] f32.

Strategy: data-parallel over batch across 8 NeuronCores (4 batches/core),
inputs converted to bf16 on the host (halves the HBM traffic; rel-err
budget 2e-2 >> bf16's ~5e-3).

Per core the routing loop is algebraically factored so u_hat [S, N*C]
is never materialized:
    iter0:   m0[(nc)]   = xsum @ W             (xsum = col-sum of x)
    V[d,n]   = sum_c W[d,(n c)] * mnorm[n,c]   (Wt-chunk @ Mblk, ap=16)
    b[s,n]   = x @ V                           (xT-chunk @ V,     ap=16)
    c        = softmax_n(b)
    G[n,d]   = c^T @ x                         (x-chunk @ c,      ap=16)
    m[(nc)]  = diag_n(W^T G)                   (W-chunk @ G^T,    ap=16)
    squash: inv = exp(-0.5 ln(|m|^2 + eps))    (one ACT table: exp+ln)
All routing matmuls keep the tiny capsule dim (16) as the moving side, so
PE streaming cost is ~16 cycles/matmul; the only large PE work is the
x-transposes (needed for the d-major contraction in b = x @ V).
"""
import numpy as np
import concourse.bass as bass
import concourse.mybir as mybir
import concourse.tile as tile
from concourse import bacc
from concourse.bass import ts, ds
from concourse.bass_utils import run_bass_kernel_spmd
from concourse.tile import add_dep_helper

F32 = mybir.dt.float32
U32 = mybir.dt.uint32
BF16 = mybir.dt.bfloat16
AF = mybir.ActivationFunctionType
AX = mybir.AxisListType
OP = mybir.AluOpType

NCORES = 8
B, S, D = 32, 512, 768
N, C = 16, 32
NC = N * C            # 512
BL = B // NCORES      # 4 batches per core
EPS = 1e-7
SCN = S // 128        # 4 s-chunks
DCN = D // 128        # 6 d-chunks
KCN = NC // 128       # 4 nc-chunks
ROUTINGS = 3
PE_WARM = 0   # dummy PE transposes chained to hold the p-state at 2.4 GHz

# const tile column layout (all bf16)
CID = 0               # [128, 128] identity (PE transposes)
CMASK = 128           # [128, 256] diag mask[(nl,c), (b,kc,n)] = (n == 4*kc+nl)
CSEL = 384            # [128, 4]   sel[p, j] = (p//32 == j)
CONE = 388            # [128, 1]   ones
CBS = 392             # [128, 128] bsel[k, m] = (k//32 == m//32)
CONW = 520


def _build_module():
    nc = bacc.Bacc("TRN2", target_bir_lowering=False, num_devices=NCORES)
    X = nc.dram_tensor("x", [BL, S, D], BF16, kind="ExternalInput")
    W = nc.dram_tensor("w", [D, NC], BF16, kind="ExternalInput")
    CON = nc.dram_tensor("consts", [128, CONW], BF16, kind="ExternalInput")
    OUT = nc.dram_tensor("out", [BL, N, C], F32, kind="ExternalOutput")

    cp_flip = [0]

    with tile.TileContext(nc) as tc:
        with (
            tc.tile_pool(name="const", bufs=1) as pc,
            tc.tile_pool(name="rt", bufs=2) as prt,
            tc.tile_pool(name="pmm", bufs=1, space="PSUM") as pmm,
            tc.tile_pool(name="ptr", bufs=2, space="PSUM") as ptr,
        ):
            def cpd(dst, src):
                return nc.vector.tensor_copy(dst, src)

            def cpa(dst, src):
                return nc.scalar.copy(dst, src)

            # ---- persistent tiles ----
            con = pc.tile([128, CONW], BF16, tag="con")
            wsb = pc.tile([128, DCN, NC], BF16, tag="w")
            wtsb = pc.tile([128, KCN, D], BF16, tag="wt")
            xsumb = pc.tile([128, BL * DCN], BF16, tag="xsum")
            epst = pc.tile([128, 1], F32, tag="eps")
            nc.vector.memset(epst[:], EPS)
            magict = pc.tile([128, 16], U32, tag="magic")
            nc.vector.memset(magict[:], 0x5F3759DF)

            def prefetch_act(func):
                # dummy [1,1] activation hoists the ACT table load early
                dum = prt.tile([1, 1], F32, tag="dum")
                nc.scalar.activation(dum[:], epst[0:1, :], func)

            # consts ride the ACT queue; x batches + W ride the SP queue.
            # W sits between x1 and x2 so WT transposes clear the PE early;
            # the last batch arrives in s-chunks so stage A can track it.
            nc.scalar.dma_start(con[:], CON[:, :])
            xbs = [pc.tile([128, SCN, D], BF16, tag=f"xb{b}", name=f"xb_{b}")
                   for b in range(BL)]
            nc.sync.dma_start(
                xbs[0][:], X[0, :, :].rearrange("(sc p) d -> p sc d", p=128)
            )
            nc.sync.dma_start(
                wsb[:], W[:, :].rearrange("(dc p) n -> p dc n", p=128)
            )
            for b in (1, 2):
                nc.sync.dma_start(
                    xbs[b][:],
                    X[b, :, :].rearrange("(sc p) d -> p sc d", p=128),
                )
            for sc in range(SCN):
                nc.sync.dma_start(
                    xbs[3][:, sc, :], X[3, ds(sc * 128, 128), :]
                )
            prefetch_act(AF.Exp)

            ident = con[:, CID:CID + 128]

            # ---- stage A: xT + xsum per batch; WT between b2 and b3 so
            # the W transposes fill the DMA wait for the last batch ----
            pxs = pmm.tile([128, BL * DCN], F32, tag="seq")
            xts = [pc.tile([128, DCN, S], BF16, tag=f"xt{b}", name=f"xt_{b}")
                   for b in range(BL)]

            def stage_a(b):
                xb = xbs[b]
                xt = xts[b]
                for j in range(DCN // 2):
                    pxt = ptr.tile([128, 2 * S], BF16, tag="tr")
                    for h in range(2):
                        dc = 2 * j + h
                        for sc in range(SCN):
                            nc.tensor.transpose(
                                pxt[:, ds(h * S + sc * 128, 128)],
                                xb[:, sc, ds(dc * 128, 128)],
                                ident,
                            )
                        for sc in range(SCN):
                            nc.tensor.matmul(
                                pxs[:, ds(b * DCN + dc, 1)],
                                xb[:, sc, ds(dc * 128, 128)],
                                con[:, CONE:CONE + 1],
                                start=(sc == 0),
                                stop=(sc == SCN - 1),
                            )
                    eng = cpa if j == 1 else cpd
                    eng(xt[:, 2 * j:2 * j + 2, :].rearrange(
                        "p a b -> p (a b)"), pxt[:])
                cpd(xsumb[:, ds(b * DCN, DCN)], pxs[:, ds(b * DCN, DCN)])

            def stage_a3():
                # last batch: transposes emitted s-chunk-major (chunks arrive
                # via four DMAs), xsum evacuated before the pair tiles so
                # iter0 can start the moment the last chunk lands
                xb, xt = xbs[3], xts[3]
                p3 = [ptr.tile([128, 2 * S], BF16, tag="tr3", bufs=3,
                               name=f"tr3_{j}") for j in range(3)]
                for sc in range(SCN):
                    for dc in range(DCN):
                        nc.tensor.transpose(
                            p3[dc // 2][:, ds((dc % 2) * S + sc * 128, 128)],
                            xb[:, sc, ds(dc * 128, 128)],
                            ident,
                        )
                for dc in range(DCN):
                    for sc in range(SCN):
                        nc.tensor.matmul(
                            pxs[:, ds(3 * DCN + dc, 1)],
                            xb[:, sc, ds(dc * 128, 128)],
                            con[:, CONE:CONE + 1],
                            start=(sc == 0),
                            stop=(sc == SCN - 1),
                        )
                xi = cpd(xsumb[:, ds(3 * DCN, DCN)],
                         pxs[:, ds(3 * DCN, DCN)])
                ji = cpa(xt[:, 2:4, :].rearrange("p a b -> p (a b)"),
                         p3[1][:])
                add_dep_helper(ji.ins, xi.ins, sync=False,
                               reason="xsum3 first")
                return xi, p3

            for b in range(2):
                stage_a(b)
            for kc in range(KCN):
                ptw = ptr.tile([128, 2 * S], BF16, tag="tr")
                for dc in range(DCN):
                    nc.tensor.transpose(
                        ptw[:, ts(dc, 128)],
                        wsb[:, dc, ds(kc * 128, 128)],
                        ident,
                    )
                (cpd if kc % 2 == 0 else cpa)(wtsb[:, kc, :], ptw[:, 0:768])
            stage_a(2)
            xi3, p3 = stage_a3()

            # PE warmer: a serial chain of dummy transposes keeps the PE
            # p-state ramped through the routing phase so the small matmul
            # groups run at full clock.
            if PE_WARM:
                pwm = pmm.tile([128, 128], BF16, tag="warm")
                for _ in range(PE_WARM):
                    nc.tensor.transpose(pwm[:], con[:, 0:128], ident)

            # ---- routing ----
            maskr = con[:, CMASK:CMASK + BL * KCN * N]

            def squash(pot, src_cols, it):
                """pot: psum [128, (b kc[ n])] -> returns mnorm tile.
                src_cols=1 for iter0 (pot is [128, (b kc)] = m directly)."""
                small = prt.tile([128, 16], F32, tag="m", name=f"m{it}")
                if src_cols == 1:
                    nc.vector.tensor_copy(small[:], pot[:])
                    m = small
                else:
                    pm = prt.tile([128, BL * KCN * N], F32, tag="pm")
                    nc.vector.tensor_mul(pm[:], pot[:], maskr)
                    nc.vector.tensor_reduce(
                        small[:],
                        pm[:].rearrange("p (g n) -> p g n", g=BL * KCN),
                        axis=AX.X,
                        op=OP.add,
                    )
                    m = small
                sq = prt.tile([128, 16], BF16, tag="sq", name=f"sq{it}")
                nc.vector.tensor_mul(sq[:], m[:], m[:])
                # fused capsule-group sum + broadcast: bsel[k,m]=(k//32==m//32)
                pnsq = pmm.tile([128, 16], F32, tag="seq", name=f"nsq{it}")
                nc.tensor.matmul(
                    pnsq[:],
                    con[:, CBS:CBS + 128],
                    sq[:],
                    start=True,
                    stop=True,
                )
                # rsqrt on DVE only (bit trick + 1 Newton step); keeps the
                # ACT table pinned to Exp for the whole kernel. nsq is
                # O(10..100) here so the reference's +eps is a no-op in bf16.
                y0u = prt.tile([128, 16], U32, tag="y0u", name=f"y0u{it}")
                nc.vector.tensor_scalar(
                    y0u[:], pnsq[:].bitcast(U32), 1, None,
                    OP.logical_shift_right,
                )
                nc.vector.tensor_sub(y0u[:], magict[:], y0u[:])
                y0f = y0u[:].bitcast(F32)
                t1 = prt.tile([128, 16], F32, tag="nt1", name=f"nt1{it}")
                nc.vector.tensor_mul(t1[:], pnsq[:], y0f)
                nc.vector.tensor_mul(t1[:], t1[:], y0f)
                nc.vector.tensor_scalar(t1[:], t1[:], -0.5, 1.5, OP.mult,
                                        OP.add)
                pinv = prt.tile([128, 16], F32, tag="rsq", name=f"rsq{it}")
                nc.vector.tensor_mul(pinv[:], y0f, t1[:])
                if it == ROUTINGS - 1:
                    mnorm = prt.tile([128, 16], BF16, tag="mnf", name="mnf")
                    nc.vector.tensor_mul(mnorm[:], m[:], pinv[:])
                    return mnorm, None
                # pre-scatter m over the diag mask on the idle GPSIMD engine
                # while the DVE rsqrt chain runs; fold pinv in afterwards
                mm1 = prt.tile([128, BL * KCN * N], BF16, tag="mm1",
                               name=f"mm1_{it}")
                m_bc = bass.AP(
                    tensor=m.tensor,
                    offset=m.offset,
                    ap=[m.ap[0], [1, BL * KCN], [0, N]],
                )
                nc.gpsimd.tensor_mul(
                    mm1[:].rearrange("p (g n) -> p g n", g=BL * KCN),
                    m_bc,
                    maskr.rearrange("p (g n) -> p g n", g=BL * KCN),
                )
                mblk = prt.tile([128, BL * KCN * N], BF16, tag="mblk",
                                name=f"mblk{it}")
                pi_bc = bass.AP(
                    tensor=pinv.tensor,
                    offset=pinv.offset,
                    ap=[pinv.ap[0], [1, BL * KCN], [0, N]],
                )
                mbi = nc.vector.tensor_mul(
                    mblk[:].rearrange("p (g n) -> p g n", g=BL * KCN),
                    mm1[:].rearrange("p (g n) -> p g n", g=BL * KCN),
                    pi_bc,
                )
                return mblk, mbi

            def v_and_b(mblk, it, mid=None):
                """V -> b (psum), half-batch interleaved with fully
                independent tiles per half so the evacuations overlap the
                other half's matmuls."""
                HB = BL * DCN * N // 2
                vsbs = []
                for half in range(2):
                    pv = pmm.tile([128, HB], F32, tag=f"big{half}",
                                  name=f"pv{it}_{half}")
                    for b in (0, 1) if half == 0 else (2, 3):
                        for dc in range(DCN):
                            col = ((b % 2) * DCN + dc) * N
                            for kc in range(KCN):
                                nc.tensor.matmul(
                                    pv[:, ds(col, N)],
                                    wtsb[:, kc, ds(dc * 128, 128)],
                                    mblk[:, ds((b * KCN + kc) * N, N)],
                                    start=(kc == 0),
                                    stop=(kc == KCN - 1),
                                )
                    vsb = prt.tile([128, HB], BF16, tag=f"vsb{half}",
                                   name=f"vsb{it}_{half}")
                    (nc.scalar.copy if half == 0
                     else nc.vector.tensor_copy)(vsb[:], pv[:])
                    vsbs.append(vsb)
                if mid is not None:
                    mid()
                pb = pmm.tile([128, BL * SCN * N], F32, tag="seq")
                for b in range(BL):
                    for sc in range(SCN):
                        for dc in range(DCN):
                            nc.tensor.matmul(
                                pb[:, ds((b * SCN + sc) * N, N)],
                                xts[b][:, dc, ds(sc * 128, 128)],
                                vsbs[b // 2][:, ds(((b % 2) * DCN + dc) * N, N)],
                                start=(dc == 0),
                                stop=(dc == DCN - 1),
                            )
                return pb

            # iter 0: uniform routing weights -> m0 = xsum @ W (diag blocks)
            pot0 = pmm.tile([128, BL * KCN], F32, tag="seq")
            for b in range(BL):
                for kc in range(KCN):
                    for dc in range(DCN):
                        nc.tensor.matmul(
                            pot0[:, ds(b * KCN + kc, 1)],
                            wsb[:, dc, ds(kc * 128, 128)],
                            xsumb[:, ds(b * DCN + dc, 1)],
                            start=(dc == 0),
                            stop=(dc == DCN - 1),
                        )
            mblk0, mbi0 = squash(pot0, 1, 0)
            # deferred batch-3 xt evacuations: pinned after the squash chain
            # so they don't occupy the DVE ahead of it
            for j in (0, 2):
                ei = cpd(xts[3][:, 2 * j:2 * j + 2, :].rearrange(
                    "p a b -> p (a b)"), p3[j][:])
                add_dep_helper(ei.ins, mbi0.ins, sync=False,
                               reason="xt3 after squash0")
            pb = v_and_b(mblk0, 0)

            for it in range(1, ROUTINGS):
                # softmax over n
                expb = prt.tile([128, BL * SCN * N], F32, tag="expb",
                                name=f"expb{it}")
                nc.scalar.activation(expb[:], pb[:], AF.Exp)
                zsum = prt.tile([128, BL * SCN], F32, tag="zsum",
                                name=f"zsum{it}")
                nc.vector.tensor_reduce(
                    zsum[:],
                    expb[:].rearrange("p (g n) -> p g n", g=BL * SCN),
                    axis=AX.X,
                    op=OP.add,
                )
                zrec = prt.tile([128, BL * SCN], F32, tag="zrec",
                                name=f"zrec{it}")
                nc.vector.reciprocal(zrec[:], zsum[:])
                cw = prt.tile([128, BL * SCN * N], BF16, tag="cw",
                              name=f"cw{it}")
                zr_bc = bass.AP(
                    tensor=zrec.tensor,
                    offset=zrec.offset,
                    ap=[zrec.ap[0], [1, BL * SCN], [0, N]],
                )
                nc.vector.tensor_mul(
                    cw[:].rearrange("p (g n) -> p g n", g=BL * SCN),
                    expb[:].rearrange("p (g n) -> p g n", g=BL * SCN),
                    zr_bc,
                )
                # G^T[d, n] per (b, dc), then outT[(nc), n] per (b, kc);
                # half-batch interleaved with independent tiles per half
                HB = BL * DCN * N // 2
                gsbs = []
                for half in range(2):
                    pg = pmm.tile([128, HB], F32, tag=f"big{half}",
                                  name=f"gp{it}_{half}")
                    for b in (0, 1) if half == 0 else (2, 3):
                        for dc in range(DCN):
                            col = ((b % 2) * DCN + dc) * N
                            for sc in range(SCN):
                                nc.tensor.matmul(
                                    pg[:, ds(col, N)],
                                    xbs[b][:, sc, ds(dc * 128, 128)],
                                    cw[:, ds((b * SCN + sc) * N, N)],
                                    start=(sc == 0),
                                    stop=(sc == SCN - 1),
                                )
                    gsb = prt.tile([128, HB], BF16, tag=f"gsb{half}",
                                   name=f"gsb{it}_{half}")
                    (nc.scalar.copy if half == 0
                     else nc.vector.tensor_copy)(gsb[:], pg[:])
                    gsbs.append(gsb)
                pot = pmm.tile([128, BL * KCN * N], F32, tag="seq",
                               name=f"potp{it}")
                for b in range(BL):
                    for kc in range(KCN):
                        for dc in range(DCN):
                            nc.tensor.matmul(
                                pot[:, ds((b * KCN + kc) * N, N)],
                                wsb[:, dc, ds(kc * 128, 128)],
                                gsbs[b // 2][:, ds(((b % 2) * DCN + dc) * N, N)],
                                start=(dc == 0),
                                stop=(dc == DCN - 1),
                            )
                mnorm, _ = squash(pot, N, it)
                if it < ROUTINGS - 1:
                    pb = v_and_b(mnorm, it)

            # final output: transpose to [(b kc), (nl c)] so each DMA
            # descriptor is a 512-byte contiguous DRAM run
            pfin = pmm.tile([16, 128], BF16, tag="seq")
            nc.tensor.transpose(pfin[:], mnorm[:], ident)
            fsb = prt.tile([16, 128], F32, tag="fsb")
            nc.scalar.copy(fsb[:], pfin[:])
            nc.sync.dma_start(
                OUT.rearrange("b (kc nl) c -> (b kc) (nl c)", kc=KCN, nl=4),
                fsb[:],
            )

    nc.compile()
    return nc


def _make_consts():
    import ml_dtypes
    con = np.zeros((128, CONW), dtype=np.float32)
    con[:, CID:CID + 128] = np.eye(128, dtype=np.float32)
    p = np.arange(128)
    for b in range(BL):
        for kc in range(KCN):
            for n in range(N):
                con[:, CMASK + (b * KCN + kc) * N + n] = (n == 4 * kc + p // 32)
    for j in range(4):
        con[:, CSEL + j] = (p // 32 == j)
    con[:, CONE] = 1.0
    con[:, CBS:CBS + 128] = (p[:, None] // 32 == p[None, :] // 32)
    return con.astype(ml_dtypes.bfloat16)


_NC_CACHE = []


def kernel(x: np.ndarray, W: np.ndarray) -> np.ndarray:
    import ml_dtypes
    assert x.shape == (B, S, D) and W.shape == (1, D, NC)
    if not _NC_CACHE:
        _NC_CACHE.append(_build_module())
    nc = _NC_CACHE[0]
    con = _make_consts()
    w2 = np.ascontiguousarray(W[0]).astype(ml_dtypes.bfloat16)
    xb = x.astype(ml_dtypes.bfloat16)
    in_maps = []
    for i in range(NCORES):
        m = {
            "x": np.ascontiguousarray(xb[i * BL:(i + 1) * BL]),
            "w": w2,
            "consts": con,
        }
        in_maps.append(m)
    res = run_bass_kernel_spmd(nc, in_maps, list(range(NCORES)))
    out = np.concatenate([res.results[i]["out"] for i in range(NCORES)], axis=0)
    return out.astype(np.float32)


# revision 23
# speedup vs baseline: 1.0567x; 1.0547x over previous
"""Trainium2 Bass kernel for the capsule-routing layer (nn_Caps_Layer).

Full inputs: x [32, 512, 768] f32, W [1, 768, 512] f32.
Output: [32, 16, 32] f32.

Strategy: data-parallel over batch across 8 NeuronCores (4 batches/core),
inputs converted to bf16 on the host (halves the HBM traffic; rel-err
budget 2e-2 >> bf16's ~5e-3).

Per core the routing loop is algebraically factored so u_hat [S, N*C]
is never materialized:
    iter0:   m0[(nc)]   = xsum @ W             (xsum = col-sum of x)
    V[d,n]   = sum_c W[d,(n c)] * mnorm[n,c]   (Wt-chunk @ Mblk, ap=16)
    b[s,n]   = x @ V                           (xT-chunk @ V,     ap=16)
    c        = softmax_n(b)
    G[n,d]   = c^T @ x                         (x-chunk @ c,      ap=16)
    m[(nc)]  = diag_n(W^T G)                   (W-chunk @ G^T,    ap=16)
    squash: inv = exp(-0.5 ln(|m|^2 + eps))    (one ACT table: exp+ln)
All routing matmuls keep the tiny capsule dim (16) as the moving side, so
PE streaming cost is ~16 cycles/matmul; the only large PE work is the
x-transposes (needed for the d-major contraction in b = x @ V).
"""
import numpy as np
import concourse.bass as bass
import concourse.mybir as mybir
import concourse.tile as tile
from concourse import bacc
from concourse.bass import ts, ds
from concourse.bass_utils import run_bass_kernel_spmd
from concourse.tile import add_dep_helper

F32 = mybir.dt.float32
U32 = mybir.dt.uint32
BF16 = mybir.dt.bfloat16
AF = mybir.ActivationFunctionType
AX = mybir.AxisListType
OP = mybir.AluOpType

NCORES = 8
B, S, D = 32, 512, 768
N, C = 16, 32
NC = N * C            # 512
BL = B // NCORES      # 4 batches per core
EPS = 1e-7
SCN = S // 128        # 4 s-chunks
DCN = D // 128        # 6 d-chunks
KCN = NC // 128       # 4 nc-chunks
ROUTINGS = 3
PE_WARM = 0   # dummy PE transposes chained to hold the p-state at 2.4 GHz

# const tile column layout (all bf16)
CID = 0               # [128, 128] identity (PE transposes)
CMASK = 128           # [128, 256] diag mask[(nl,c), (b,kc,n)] = (n == 4*kc+nl)
CSEL = 384            # [128, 4]   sel[p, j] = (p//32 == j)
CONE = 388            # [128, 1]   ones
CBS = 392             # [128, 128] bsel[k, m] = (k//32 == m//32)
CONW = 520


def _build_module():
    nc = bacc.Bacc("TRN2", target_bir_lowering=False, num_devices=NCORES)
    X = nc.dram_tensor("x", [BL, S, D], BF16, kind="ExternalInput")
    W = nc.dram_tensor("w", [D, NC], BF16, kind="ExternalInput")
    CON = nc.dram_tensor("consts", [128, CONW], BF16, kind="ExternalInput")
    OUT = nc.dram_tensor("out", [BL, N, C], F32, kind="ExternalOutput")

    cp_flip = [0]

    with tile.TileContext(nc) as tc:
        with (
            tc.tile_pool(name="const", bufs=1) as pc,
            tc.tile_pool(name="rt", bufs=2) as prt,
            tc.tile_pool(name="pmm", bufs=1, space="PSUM") as pmm,
            tc.tile_pool(name="ptr", bufs=5, space="PSUM") as ptr,
        ):
            def cpd(dst, src):
                return nc.vector.tensor_copy(dst, src)

            def cpa(dst, src):
                return nc.scalar.copy(dst, src)

            # ---- persistent tiles ----
            con = pc.tile([128, CONW], BF16, tag="con")
            wsb = pc.tile([128, DCN, NC], BF16, tag="w")
            wtsb = pc.tile([128, KCN, D], BF16, tag="wt")
            xsumb = pc.tile([128, BL * DCN], BF16, tag="xsum")
            epst = pc.tile([128, 1], F32, tag="eps")
            nc.vector.memset(epst[:], EPS)
            magict = pc.tile([128, 16], U32, tag="magic")
            nc.vector.memset(magict[:], 0x5F3759DF)

            def prefetch_act(func):
                # dummy [1,1] activation hoists the ACT table load early
                dum = prt.tile([1, 1], F32, tag="dum")
                nc.scalar.activation(dum[:], epst[0:1, :], func)

            # consts ride the ACT queue; x batches + W ride the SP queue.
            # W sits between x1 and x2 so WT transposes clear the PE early;
            # the last batch arrives in s-chunks so stage A can track it.
            nc.scalar.dma_start(con[:], CON[:, :])
            xbs = [pc.tile([128, SCN, D], BF16, tag=f"xb{b}", name=f"xb_{b}")
                   for b in range(BL)]
            nc.sync.dma_start(
                xbs[0][:], X[0, :, :].rearrange("(sc p) d -> p sc d", p=128)
            )
            nc.sync.dma_start(
                wsb[:], W[:, :].rearrange("(dc p) n -> p dc n", p=128)
            )
            for b in (1, 2):
                nc.sync.dma_start(
                    xbs[b][:],
                    X[b, :, :].rearrange("(sc p) d -> p sc d", p=128),
                )
            for sc in range(SCN):
                nc.sync.dma_start(
                    xbs[3][:, sc, :], X[3, ds(sc * 128, 128), :]
                )
            prefetch_act(AF.Exp)

            ident = con[:, CID:CID + 128]

            # ---- stage A: xT + xsum per batch; WT between b2 and b3 so
            # the W transposes fill the DMA wait for the last batch ----
            pxs = pmm.tile([128, BL * DCN], F32, tag="seq")
            xts = [pc.tile([128, DCN, S], BF16, tag=f"xt{b}", name=f"xt_{b}")
                   for b in range(BL)]

            def stage_a(b):
                xb = xbs[b]
                xt = xts[b]
                for j in range(DCN // 2):
                    pxt = ptr.tile([128, 2 * S], BF16, tag="tr")
                    for h in range(2):
                        dc = 2 * j + h
                        for sc in range(SCN):
                            nc.tensor.transpose(
                                pxt[:, ds(h * S + sc * 128, 128)],
                                xb[:, sc, ds(dc * 128, 128)],
                                ident,
                            )
                        for sc in range(SCN):
                            nc.tensor.matmul(
                                pxs[:, ds(b * DCN + dc, 1)],
                                xb[:, sc, ds(dc * 128, 128)],
                                con[:, CONE:CONE + 1],
                                start=(sc == 0),
                                stop=(sc == SCN - 1),
                            )
                    eng = cpa if j == 1 else cpd
                    eng(xt[:, 2 * j:2 * j + 2, :].rearrange(
                        "p a b -> p (a b)"), pxt[:])
                cpd(xsumb[:, ds(b * DCN, DCN)], pxs[:, ds(b * DCN, DCN)])

            def stage_a3():
                # last batch: transposes emitted s-chunk-major (chunks arrive
                # via four DMAs), xsum evacuated before the pair tiles so
                # iter0 can start the moment the last chunk lands
                xb, xt = xbs[3], xts[3]
                p3 = [ptr.tile([128, 2 * S], BF16, tag="tr",
                               name=f"tr3_{j}") for j in range(3)]
                for sc in range(SCN):
                    for dc in range(DCN):
                        nc.tensor.transpose(
                            p3[dc // 2][:, ds((dc % 2) * S + sc * 128, 128)],
                            xb[:, sc, ds(dc * 128, 128)],
                            ident,
                        )
                for dc in range(DCN):
                    for sc in range(SCN):
                        nc.tensor.matmul(
                            pxs[:, ds(3 * DCN + dc, 1)],
                            xb[:, sc, ds(dc * 128, 128)],
                            con[:, CONE:CONE + 1],
                            start=(sc == 0),
                            stop=(sc == SCN - 1),
                        )
                xi = cpd(xsumb[:, ds(3 * DCN, DCN)],
                         pxs[:, ds(3 * DCN, DCN)])
                ji = cpa(xt[:, 2:4, :].rearrange("p a b -> p (a b)"),
                         p3[1][:])
                add_dep_helper(ji.ins, xi.ins, sync=False,
                               reason="xsum3 first")
                return xi, p3

            for b in range(2):
                stage_a(b)
            for kc in range(KCN):
                ptw = ptr.tile([128, 2 * S], BF16, tag="tr")
                for dc in range(DCN):
                    nc.tensor.transpose(
                        ptw[:, ts(dc, 128)],
                        wsb[:, dc, ds(kc * 128, 128)],
                        ident,
                    )
                (cpd if kc % 2 == 0 else cpa)(wtsb[:, kc, :], ptw[:, 0:768])
            stage_a(2)
            xi3, p3 = stage_a3()

            # PE warmer: a serial chain of dummy transposes keeps the PE
            # p-state ramped through the routing phase so the small matmul
            # groups run at full clock.
            if PE_WARM:
                pwm = pmm.tile([128, 128], BF16, tag="warm")
                for _ in range(PE_WARM):
                    nc.tensor.transpose(pwm[:], con[:, 0:128], ident)

            # ---- routing ----
            maskr = con[:, CMASK:CMASK + BL * KCN * N]

            def squash(pot, src_cols, it):
                """pot: psum [128, (b kc[ n])] -> returns mnorm tile.
                src_cols=1 for iter0 (pot is [128, (b kc)] = m directly)."""
                small = prt.tile([128, 16], F32, tag="m", name=f"m{it}")
                if src_cols == 1:
                    nc.vector.tensor_copy(small[:], pot[:])
                    m = small
                else:
                    pm = prt.tile([128, BL * KCN * N], F32, tag="pm")
                    nc.vector.tensor_mul(pm[:], pot[:], maskr)
                    nc.vector.tensor_reduce(
                        small[:],
                        pm[:].rearrange("p (g n) -> p g n", g=BL * KCN),
                        axis=AX.X,
                        op=OP.add,
                    )
                    m = small
                sq = prt.tile([128, 16], BF16, tag="sq", name=f"sq{it}")
                nc.vector.tensor_mul(sq[:], m[:], m[:])
                # fused capsule-group sum + broadcast: bsel[k,m]=(k//32==m//32)
                pnsq = pmm.tile([128, 16], F32, tag="seq", name=f"nsq{it}")
                nc.tensor.matmul(
                    pnsq[:],
                    con[:, CBS:CBS + 128],
                    sq[:],
                    start=True,
                    stop=True,
                )
                # rsqrt on DVE only (bit trick + 1 Newton step); keeps the
                # ACT table pinned to Exp for the whole kernel. nsq is
                # O(10..100) here so the reference's +eps is a no-op in bf16.
                y0u = prt.tile([128, 16], U32, tag="y0u", name=f"y0u{it}")
                nc.vector.tensor_scalar(
                    y0u[:], pnsq[:].bitcast(U32), 1, None,
                    OP.logical_shift_right,
                )
                nc.vector.tensor_sub(y0u[:], magict[:], y0u[:])
                y0f = y0u[:].bitcast(F32)
                t1 = prt.tile([128, 16], F32, tag="nt1", name=f"nt1{it}")
                nc.vector.tensor_mul(t1[:], pnsq[:], y0f)
                nc.vector.tensor_mul(t1[:], t1[:], y0f)
                nc.vector.tensor_scalar(t1[:], t1[:], -0.5, 1.5, OP.mult,
                                        OP.add)
                pinv = prt.tile([128, 16], F32, tag="rsq", name=f"rsq{it}")
                nc.vector.tensor_mul(pinv[:], y0f, t1[:])
                if it == ROUTINGS - 1:
                    mnorm = prt.tile([128, 16], BF16, tag="mnf", name="mnf")
                    nc.vector.tensor_mul(mnorm[:], m[:], pinv[:])
                    return mnorm, None
                # pre-scatter m over the diag mask on the idle GPSIMD engine
                # while the DVE rsqrt chain runs; fold pinv in afterwards
                mm1 = prt.tile([128, BL * KCN * N], BF16, tag="mm1",
                               name=f"mm1_{it}")
                m_bc = bass.AP(
                    tensor=m.tensor,
                    offset=m.offset,
                    ap=[m.ap[0], [1, BL * KCN], [0, N]],
                )
                nc.gpsimd.tensor_mul(
                    mm1[:].rearrange("p (g n) -> p g n", g=BL * KCN),
                    m_bc,
                    maskr.rearrange("p (g n) -> p g n", g=BL * KCN),
                )
                mblk = prt.tile([128, BL * KCN * N], BF16, tag="mblk",
                                name=f"mblk{it}")
                pi_bc = bass.AP(
                    tensor=pinv.tensor,
                    offset=pinv.offset,
                    ap=[pinv.ap[0], [1, BL * KCN], [0, N]],
                )
                mbi = nc.vector.tensor_mul(
                    mblk[:].rearrange("p (g n) -> p g n", g=BL * KCN),
                    mm1[:].rearrange("p (g n) -> p g n", g=BL * KCN),
                    pi_bc,
                )
                return mblk, mbi

            def v_and_b(mblk, it, mid=None):
                """V -> b (psum), half-batch interleaved with fully
                independent tiles per half so the evacuations overlap the
                other half's matmuls."""
                HB = BL * DCN * N // 2
                vsbs = []
                for half in range(2):
                    pv = pmm.tile([128, HB], F32, tag=f"big{half}",
                                  name=f"pv{it}_{half}")
                    for b in (0, 1) if half == 0 else (2, 3):
                        for dc in range(DCN):
                            col = ((b % 2) * DCN + dc) * N
                            for kc in range(KCN):
                                nc.tensor.matmul(
                                    pv[:, ds(col, N)],
                                    wtsb[:, kc, ds(dc * 128, 128)],
                                    mblk[:, ds((b * KCN + kc) * N, N)],
                                    start=(kc == 0),
                                    stop=(kc == KCN - 1),
                                )
                    vsb = prt.tile([128, HB], BF16, tag=f"vsb{half}",
                                   name=f"vsb{it}_{half}")
                    (nc.scalar.copy if half == 0
                     else nc.vector.tensor_copy)(vsb[:], pv[:])
                    vsbs.append(vsb)
                if mid is not None:
                    mid()
                pb = pmm.tile([128, BL * SCN * N], F32, tag="seq")
                for b in range(BL):
                    for sc in range(SCN):
                        for dc in range(DCN):
                            nc.tensor.matmul(
                                pb[:, ds((b * SCN + sc) * N, N)],
                                xts[b][:, dc, ds(sc * 128, 128)],
                                vsbs[b // 2][:, ds(((b % 2) * DCN + dc) * N, N)],
                                start=(dc == 0),
                                stop=(dc == DCN - 1),
                            )
                return pb

            # iter 0: uniform routing weights -> m0 = xsum @ W (diag blocks)
            pot0 = pmm.tile([128, BL * KCN], F32, tag="seq")
            for b in range(BL):
                for kc in range(KCN):
                    for dc in range(DCN):
                        nc.tensor.matmul(
                            pot0[:, ds(b * KCN + kc, 1)],
                            wsb[:, dc, ds(kc * 128, 128)],
                            xsumb[:, ds(b * DCN + dc, 1)],
                            start=(dc == 0),
                            stop=(dc == DCN - 1),
                        )
            mblk0, mbi0 = squash(pot0, 1, 0)
            # deferred batch-3 xt evacuations: pinned after the squash chain
            # so they don't occupy the DVE ahead of it
            for j in (0, 2):
                ei = cpd(xts[3][:, 2 * j:2 * j + 2, :].rearrange(
                    "p a b -> p (a b)"), p3[j][:])
                add_dep_helper(ei.ins, mbi0.ins, sync=False,
                               reason="xt3 after squash0")
            pb = v_and_b(mblk0, 0)

            for it in range(1, ROUTINGS):
                # softmax over n
                expb = prt.tile([128, BL * SCN * N], F32, tag="expb",
                                name=f"expb{it}")
                nc.scalar.activation(expb[:], pb[:], AF.Exp)
                zsum = prt.tile([128, BL * SCN], F32, tag="zsum",
                                name=f"zsum{it}")
                nc.vector.tensor_reduce(
                    zsum[:],
                    expb[:].rearrange("p (g n) -> p g n", g=BL * SCN),
                    axis=AX.X,
                    op=OP.add,
                )
                zrec = prt.tile([128, BL * SCN], F32, tag="zrec",
                                name=f"zrec{it}")
                nc.vector.reciprocal(zrec[:], zsum[:])
                cw = prt.tile([128, BL * SCN * N], BF16, tag="cw",
                              name=f"cw{it}")
                zr_bc = bass.AP(
                    tensor=zrec.tensor,
                    offset=zrec.offset,
                    ap=[zrec.ap[0], [1, BL * SCN], [0, N]],
                )
                nc.vector.tensor_mul(
                    cw[:].rearrange("p (g n) -> p g n", g=BL * SCN),
                    expb[:].rearrange("p (g n) -> p g n", g=BL * SCN),
                    zr_bc,
                )
                # G^T[d, n] per (b, dc), then outT[(nc), n] per (b, kc);
                # half-batch interleaved with independent tiles per half
                HB = BL * DCN * N // 2
                gsbs = []
                for half in range(2):
                    pg = pmm.tile([128, HB], F32, tag=f"big{half}",
                                  name=f"gp{it}_{half}")
                    for b in (0, 1) if half == 0 else (2, 3):
                        for dc in range(DCN):
                            col = ((b % 2) * DCN + dc) * N
                            for sc in range(SCN):
                                nc.tensor.matmul(
                                    pg[:, ds(col, N)],
                                    xbs[b][:, sc, ds(dc * 128, 128)],
                                    cw[:, ds((b * SCN + sc) * N, N)],
                                    start=(sc == 0),
                                    stop=(sc == SCN - 1),
                                )
                    gsb = prt.tile([128, HB], BF16, tag=f"gsb{half}",
                                   name=f"gsb{it}_{half}")
                    (nc.scalar.copy if half == 0
                     else nc.vector.tensor_copy)(gsb[:], pg[:])
                    gsbs.append(gsb)
                pot = pmm.tile([128, BL * KCN * N], F32, tag="seq",
                               name=f"potp{it}")
                for b in range(BL):
                    for kc in range(KCN):
                        for dc in range(DCN):
                            nc.tensor.matmul(
                                pot[:, ds((b * KCN + kc) * N, N)],
                                wsb[:, dc, ds(kc * 128, 128)],
                                gsbs[b // 2][:, ds(((b % 2) * DCN + dc) * N, N)],
                                start=(dc == 0),
                                stop=(dc == DCN - 1),
                            )
                mnorm, _ = squash(pot, N, it)
                if it < ROUTINGS - 1:
                    pb = v_and_b(mnorm, it)

            # final output: transpose to [(b kc), (nl c)] so each DMA
            # descriptor is a 512-byte contiguous DRAM run
            pfin = pmm.tile([16, 128], BF16, tag="seq")
            nc.tensor.transpose(pfin[:], mnorm[:], ident)
            fsb = prt.tile([16, 128], F32, tag="fsb")
            nc.scalar.copy(fsb[:], pfin[:])
            nc.sync.dma_start(
                OUT.rearrange("b (kc nl) c -> (b kc) (nl c)", kc=KCN, nl=4),
                fsb[:],
            )

    nc.compile()
    return nc


def _make_consts():
    import ml_dtypes
    con = np.zeros((128, CONW), dtype=np.float32)
    con[:, CID:CID + 128] = np.eye(128, dtype=np.float32)
    p = np.arange(128)
    for b in range(BL):
        for kc in range(KCN):
            for n in range(N):
                con[:, CMASK + (b * KCN + kc) * N + n] = (n == 4 * kc + p // 32)
    for j in range(4):
        con[:, CSEL + j] = (p // 32 == j)
    con[:, CONE] = 1.0
    con[:, CBS:CBS + 128] = (p[:, None] // 32 == p[None, :] // 32)
    return con.astype(ml_dtypes.bfloat16)


_NC_CACHE = []


def kernel(x: np.ndarray, W: np.ndarray) -> np.ndarray:
    import ml_dtypes
    assert x.shape == (B, S, D) and W.shape == (1, D, NC)
    if not _NC_CACHE:
        _NC_CACHE.append(_build_module())
    nc = _NC_CACHE[0]
    con = _make_consts()
    w2 = np.ascontiguousarray(W[0]).astype(ml_dtypes.bfloat16)
    xb = x.astype(ml_dtypes.bfloat16)
    in_maps = []
    for i in range(NCORES):
        m = {
            "x": np.ascontiguousarray(xb[i * BL:(i + 1) * BL]),
            "w": w2,
            "consts": con,
        }
        in_maps.append(m)
    res = run_bass_kernel_spmd(nc, in_maps, list(range(NCORES)))
    out = np.concatenate([res.results[i]["out"] for i in range(NCORES)], axis=0)
    return out.astype(np.float32)
